# revision 1
# baseline (speedup 1.0000x reference)
"""Trainium2 Bass kernel for nn_Classifier (GNN message passing, 800 graphs x 100 nodes).

Strategy: graphs are data-parallel across 8 NeuronCores (100 graphs/core).
Each core builds its graphs' dense adjacency A^T on device via one-hot
outer-product matmuls (one-hots generated on DVE with iota+is_equal),
then every GraphConv is a dense per-graph matmul. BatchNorm statistics
are exchanged with two small AllReduces (as second moments, computed
pre-weight via M2 = z^T z so the stat reduction is matmul work); the
graph-level readout is AllGathered and the tiny [800,128] tail is
replicated on every core.

Host-side work is data reorganization only: edges are grouped by graph
(argsort), converted to graph-local ids, padded, and laid out for the
device; all arithmetic on values happens on the NeuronCores.
"""
import os
import sys

sys.path.insert(0, '/opt/trn_rl_repo')

import numpy as np
import ml_dtypes

NC_CORES = 8
B = 800          # graphs
NPG = 100        # nodes per graph
GPC = B // NC_CORES   # graphs per core
NTOT = B * NPG
E = 640000
EPG = 1024       # padded edges per graph
CH = EPG // 128  # edge chunks per graph
HID = 128
IND = 20
NCLS = 10
EPS = 1e-5
SENT = 200       # pad sentinel: never matches iota 0..127

F32 = None
BF16 = None


def _host_prep(src, dst):
    """Group edges by graph, convert to local ids, pad, chunk-layout per core."""
    src = np.asarray(src).astype(np.int64)
    dst = np.asarray(dst).astype(np.int64)
    eg = src // NPG
    assert ((dst // NPG) == eg).all(), "edges must stay within a graph"
    order = np.argsort(eg, kind='stable')
    counts = np.bincount(eg, minlength=B)
    assert counts.max() <= EPG, f"graph with {counts.max()} edges > EPG={EPG}"
    offs = np.zeros(B + 1, np.int64)
    np.cumsum(counts, out=offs[1:])
    ego = eg[order]
    rank = np.arange(E, dtype=np.int64) - offs[ego]
    srcL = np.full((B, EPG), SENT, np.int32)
    dstL = np.full((B, EPG), SENT, np.int32)
    srcL[ego, rank] = src[order] % NPG
    dstL[ego, rank] = dst[order] % NPG
    per_core = []
    for c in range(NC_CORES):
        sl = srcL[c * GPC:(c + 1) * GPC].reshape(GPC, CH, 128)
        dl = dstL[c * GPC:(c + 1) * GPC].reshape(GPC, CH, 128)
        sl = np.ascontiguousarray(sl.transpose(2, 0, 1).reshape(128, GPC * CH))
        dl = np.ascontiguousarray(dl.transpose(2, 0, 1).reshape(128, GPC * CH))
        per_core.append((sl.astype(np.float32), dl.astype(np.float32)))
    return per_core


def _build_program(bv_val):
    DBG = bool(int(os.environ.get("GNN_KERNEL_DEBUG", "0")))
    import concourse.bass as bass
    import concourse.bacc as bacc
    import concourse.tile as tile
    from concourse import mybir

    global F32, BF16
    F32 = mybir.dt.float32
    BF16 = mybir.dt.float16   # 2-byte dtype: fp16 mantissa needed for readout precision
    EQ = mybir.AluOpType.is_equal
    MUL = mybir.AluOpType.mult
    ADD = mybir.AluOpType.add
    SUB = mybir.AluOpType.subtract
    AF = mybir.ActivationFunctionType

    nc = bacc.Bacc("TRN2", target_bir_lowering=False, debug=False,
                   num_devices=NC_CORES)

    # ---------------- DRAM I/O ----------------
    d_srcL = nc.dram_tensor("srcL", [128, GPC * CH], F32, kind="ExternalInput")
    d_dstL = nc.dram_tensor("dstL", [128, GPC * CH], F32, kind="ExternalInput")
    d_W1 = nc.dram_tensor("W1", [IND, HID], F32, kind="ExternalInput")
    d_W2 = nc.dram_tensor("W2", [HID, HID], F32, kind="ExternalInput")
    d_Wg = nc.dram_tensor("Wg", [HID, HID], F32, kind="ExternalInput")
    d_Wc = nc.dram_tensor("Wc", [HID, NCLS], F32, kind="ExternalInput")
    d_P = nc.dram_tensor("P", [128, 8], F32, kind="ExternalInput")
    d_out = nc.dram_tensor("out", [B, NCLS], F32, kind="ExternalOutput")
    d_dbg = nc.dram_tensor("dbg", [128, 3072], F32, kind="ExternalOutput") if DBG else None

    # collective bounce buffers
    d_warm_i = nc.dram_tensor("warm_i", [32, 32], F32)
    d_warm_o = nc.dram_tensor("warm_o", [32, 32], F32, addr_space="Shared")
    d_ar1_i = nc.dram_tensor("ar1_i", [32, 32], F32)
    d_ar1_o = nc.dram_tensor("ar1_o", [32, 32], F32, addr_space="Shared")
    d_ar2_i = nc.dram_tensor("ar2_i", [HID, HID + 1], F32)
    d_ar2_o = nc.dram_tensor("ar2_o", [HID, HID + 1], F32, addr_space="Shared")
    d_ag_i = nc.dram_tensor("ag_i", [HID, GPC], F32)
    d_ag_o = nc.dram_tensor("ag_o", [NC_CORES * HID, GPC], F32, addr_space="Shared")

    RG = [list(range(NC_CORES))]

    with tile.TileContext(nc) as tc:
        with tc.tile_pool(name="persist", bufs=1) as pp, \
             tc.tile_pool(name="work", bufs=4) as wp, \
             tc.tile_pool(name="nm", bufs=2) as nmp, \
             tc.tile_pool(name="fm", bufs=2) as fmp:

            # ---------------- setup: constants, params ----------------
            it_i = pp.tile([128, 128], mybir.dt.int32, tag="it_i")
            nc.gpsimd.iota(it_i[:], pattern=[[1, 128]], base=0, channel_multiplier=0)
            iota_f = pp.tile([128, 128], F32, tag="iota_f")
            nc.vector.tensor_copy(iota_f[:], it_i[:])
            pc_i = pp.tile([128, 1], mybir.dt.int32, tag="pc_i")
            nc.gpsimd.iota(pc_i[:], pattern=[[1, 1]], base=0, channel_multiplier=1)
            pcol = pp.tile([128, 1], F32, tag="pcol")
            nc.vector.tensor_copy(pcol[:], pc_i[:])
            identb = pp.tile([128, 128], BF16, tag="identb")
            nc.vector.tensor_scalar(identb[:], iota_f[:], pcol[:], None, op0=EQ)
            identf = pp.tile([128, 128], F32, tag="identf")
            nc.vector.tensor_scalar(identf[:], iota_f[:], pcol[:], None, op0=EQ)
            iota_b = pp.tile([128, 128], BF16, tag="iota_b")
            nc.vector.tensor_copy(iota_b[:], iota_f[:])

            ones_b = pp.tile([128, 1], BF16, tag="ones_b")
            nc.vector.memset(ones_b[:], 1.0)
            ones_f = pp.tile([128, 1], F32, tag="ones_f")
            nc.vector.memset(ones_f[:], 1.0)
            onesrow_f = pp.tile([1, 128], F32, tag="onesrow_f")
            nc.vector.memset(onesrow_f[:], 1.0)
            negone_f = pp.tile([128, 1], F32, tag="negone_f")
            nc.vector.memset(negone_f[:], -1.0)
            negones_b = pp.tile([100, 128], BF16, tag="negones_b")
            nc.vector.memset(negones_b[:], -1.0)

            srcl = pp.tile([128, GPC * CH], F32, tag="srcl")
            nc.sync.dma_start(srcl[:], d_srcL[:])
            dstl = pp.tile([128, GPC * CH], F32, tag="dstl")
            nc.sync.dma_start(dstl[:], d_dstL[:])

            W1f = pp.tile([IND, HID], F32, tag="W1f")
            nc.sync.dma_start(W1f[:], d_W1[:])
            W2f = pp.tile([HID, HID], F32, tag="W2f")
            nc.sync.dma_start(W2f[:], d_W2[:])
            Wgf = pp.tile([HID, HID], F32, tag="Wgf")
            nc.sync.dma_start(Wgf[:], d_Wg[:])
            Wcf = pp.tile([HID, NCLS], F32, tag="Wcf")
            nc.sync.dma_start(Wcf[:], d_Wc[:])
            Pm = pp.tile([128, 8], F32, tag="Pm")
            nc.sync.dma_start(Pm[:], d_P[:])

            W1b = pp.tile([IND, HID], BF16, tag="W1b")
            nc.vector.tensor_copy(W1b[:], W1f[:])
            W2b = pp.tile([HID, HID], BF16, tag="W2b")
            nc.vector.tensor_copy(W2b[:], W2f[:])
            Wgb = pp.tile([HID, HID], BF16, tag="Wgb")
            nc.vector.tensor_copy(Wgb[:], Wgf[:])
            Wcb = pp.tile([HID, NCLS], BF16, tag="Wcb")
            nc.vector.tensor_copy(Wcb[:], Wcf[:])
            Wvf = Pm[:, 7:8]                       # [128,1] f32
            Wvb = pp.tile([128, 1], BF16, tag="Wvb")
            nc.vector.tensor_copy(Wvb[:], Wvf)

            g1c, be1c = Pm[:, 0:1], Pm[:, 1:2]
            g2c, be2c = Pm[:, 2:3], Pm[:, 3:4]
            g3c, be3c = Pm[:, 4:5], Pm[:, 5:6]
            bcc = Pm[0:NCLS, 6:7]

            # warmup collective (absorbs SPMD launch skew, warms CC path)
            warm_s = wp.tile([32, 32], F32, tag="warm")
            nc.gpsimd.memset(warm_s[:], 0.0)
            nc.gpsimd.dma_start(d_warm_i[:], warm_s[:])
            nc.gpsimd.collective_compute(
                "AllReduce", ADD, replica_groups=RG,
                ins=[d_warm_i.ap()], outs=[d_warm_o.ap()])

            dbg = None
            if DBG:
                dbg = pp.tile([128, 3072], F32, tag="dbg")
                nc.vector.memset(dbg[:], 0.0)

            # ---------------- persistent SBUF state ----------------
            AT = pp.tile([100, GPC * 128], BF16, tag="AT")       # A^T per graph (128-stride, zero-padded)
            nc.vector.memset(AT[:], 0.0)
            x0 = nmp.tile([100, GPC * IND], BF16, tag="nm")
            agg1s = nmp.tile([100, GPC * IND], BF16, tag="nm")
            scr1 = pp.tile([128, GPC * 100], BF16, tag="scr1")

            sm = pp.tile([100, 16 * 100], F32, tag="sm")  # small per-graph stats, 100-col blocks
            outdeg = sm[:, 0:100]
            indeg = sm[:, 100:200]
            invout = sm[:, 200:300]
            invin = sm[:, 300:400]
            degc = sm[:, 400:500]
            kaps = sm[:, 500:600]
            qt = sm[:, 600:700]
            tmpA = sm[:, 700:800]
            tmpB = sm[:, 800:900]
            invoutb = pp.tile([100, 100], BF16, tag="invoutb")
            qtb = pp.tile([100, 100], BF16, tag="qtb")
            dbf = pp.tile([100, 100], BF16, tag="dbf")

            col = pp.tile([128, 24], F32, tag="col")  # per-channel columns
            SzC, Mu1, Ep1, Va1, A1, C1, NA1, NC1 = (col[:, i:i + 1] for i in range(8))
            Mu2, Ep2, Va2, A2, C2, NA2, NC2 = (col[:, i:i + 1] for i in range(8, 15))
            MuV, Sz3, Sq3, A3, C3, TA, TB, TC = (col[:, i:i + 1] for i in range(15, 23))

            # ============ P1: A^T build ============
            with tc.tile_pool(name="pA", bufs=3, space="PSUM") as pA:
                for gb in range(GPC // 4):
                    ps = pA.tile([128, 512], F32, tag="ps")
                    for gi in range(4):
                        g = gb * 4 + gi
                        for ch in range(CH):
                            colix = g * CH + ch
                            ohs = wp.tile([128, 128], BF16, tag="ohs")
                            nc.vector.tensor_scalar(
                                ohs[:], iota_b[:], srcl[:, colix:colix + 1], None, op0=EQ)
                            ohd = wp.tile([128, 128], BF16, tag="ohd")
                            nc.vector.tensor_scalar(
                                ohd[:], iota_b[:], dstl[:, colix:colix + 1], None, op0=EQ)
                            nc.tensor.matmul(
                                ps[:, gi * 128:(gi + 1) * 128], ohs[:], ohd[:],
                                start=(ch == 0), stop=(ch == CH - 1))
                    nc.scalar.copy(AT[:, gb * 512:(gb + 1) * 512], ps[0:100, :])

            # ============ P2: degrees ============
            at3 = AT[:].rearrange("p (g d) -> p g d", d=128)
            nc.vector.reduce_sum(outdeg, at3, axis=mybir.AxisListType.X)
            with tc.tile_pool(name="pD", bufs=1, space="PSUM") as pD:
                psd = pD.tile([128, 100], F32, tag="psd")
                for g in range(GPC):
                    nc.tensor.matmul(psd[:, g:g + 1],
                                     AT[:, g * 128:(g + 1) * 128], ones_b[0:100, :],
                                     start=True, stop=True)
                nc.scalar.copy(indeg, psd[0:100, :])

            nc.vector.tensor_scalar(tmpA, outdeg, 1.0, None, op0=mybir.AluOpType.max)
            nc.vector.reciprocal(tmpB, tmpA)
            nc.scalar.sqrt(invout, tmpB)
            nc.vector.tensor_scalar(tmpA, indeg, 1.0, None, op0=mybir.AluOpType.max)
            nc.vector.reciprocal(tmpB, tmpA)
            nc.scalar.sqrt(invin, tmpB)
            nc.vector.tensor_scalar(degc, indeg, float(IND - 1), None,
                                    op0=mybir.AluOpType.min)
            nc.vector.tensor_copy(invoutb[:], invout)

            # ============ P3: x0 = onehot(degc) * invout ============
            for g in range(GPC):
                nc.vector.tensor_scalar(
                    x0[:, g * IND:(g + 1) * IND], iota_b[0:100, 0:IND],
                    degc[:, g:g + 1], invout[:, g:g + 1], op0=EQ, op1=MUL)

            if DBG:
                nc.vector.tensor_copy(dbg[0:100, 656:696], x0[:, 0:40])
            # ============ P4: conv1 message passing ============
            with tc.tile_pool(name="pC1", bufs=2, space="PSUM") as pC1:
                for gb in range(GPC // 25):
                    ps = pC1.tile([128, 500], F32, tag="ps")
                    for gi in range(25):
                        g = gb * 25 + gi
                        nc.tensor.matmul(ps[:, gi * IND:(gi + 1) * IND],
                                         AT[:, g * 128:(g + 1) * 128],
                                         x0[:, g * IND:(g + 1) * IND],
                                         start=True, stop=True)
                    for gi in range(25):
                        g = gb * 25 + gi
                        nc.scalar.activation(
                            agg1s[:, g * IND:(g + 1) * IND],
                            ps[0:100, gi * IND:(gi + 1) * IND],
                            AF.Identity, scale=invin[:, g:g + 1])

            if DBG:
                nc.vector.tensor_copy(dbg[0:100, 696:736], agg1s[:, 0:40])
            agg1T = fmp.tile([128, GPC * 100], BF16, tag="fm")
            # ============ P5: stats (Sz, M2) + transpose of agg1s ============
            with tc.tile_pool(name="pS1", bufs=1, space="PSUM") as pS1, \
                 tc.tile_pool(name="pT1", bufs=2, space="PSUM") as pT1:
                psz = pS1.tile([IND, 1], F32, tag="psz")
                pm2 = pS1.tile([IND, IND], F32, tag="pm2")
                for gb in range(GPC // 5):
                    pt = pT1.tile([IND, 500], F32, tag="pt")
                    for gi in range(5):
                        g = gb * 5 + gi
                        a = agg1s[:, g * IND:(g + 1) * IND]
                        nc.tensor.matmul(pt[:, gi * 100:(gi + 1) * 100], a,
                                         identb[0:100, 0:100], start=True, stop=True)
                        nc.tensor.matmul(psz[:], a, ones_b[0:100, :],
                                         start=(g == 0), stop=(g == GPC - 1))
                        nc.tensor.matmul(pm2[:], a, a,
                                         start=(g == 0), stop=(g == GPC - 1))
                    nc.scalar.copy(agg1T[0:IND, gb * 500:(gb + 1) * 500], pt[:])
                st1 = wp.tile([32, 32], F32, tag="st1")
                nc.vector.memset(st1[:], 0.0)
                nc.scalar.copy(st1[0:IND, 0:1], psz[:])
                nc.scalar.copy(st1[0:IND, 1:IND + 1], pm2[:])
                nc.gpsimd.dma_start(d_ar1_i[:], st1[:])

            # kappa = invin * (A^T.T @ invout)  (independent of h1; overlaps AR1)
            with tc.tile_pool(name="pK", bufs=1, space="PSUM") as pK:
                psk = pK.tile([128, 100], F32, tag="psk")
                for g in range(GPC):
                    nc.tensor.matmul(psk[:, g:g + 1],
                                     AT[:, g * 128:(g + 1) * 128],
                                     invoutb[:, g:g + 1],
                                     start=True, stop=True)
                nc.vector.tensor_tensor(kaps, psk[0:100, :], invin, op=MUL)
                nc.vector.tensor_scalar(kaps, kaps, -1.0, None, op0=MUL)  # negated

            # ============ AR1 + BN1 coefficient computation ============
            nc.gpsimd.collective_compute(
                "AllReduce", ADD, replica_groups=RG,
                ins=[d_ar1_i.ap()], outs=[d_ar1_o.ap()])
            st1r = wp.tile([32, 32], F32, tag="st1r")
            nc.gpsimd.dma_start(st1r[:], d_ar1_o[:])

            with tc.tile_pool(name="pB1", bufs=1, space="PSUM") as pB1:
                pmu = pB1.tile([128, 1], F32, tag="pmu")
                nc.tensor.matmul(pmu[:], W1f[:], st1r[0:IND, 0:1], start=True, stop=True)
                nc.scalar.mul(Mu1, pmu[:], 1.0 / NTOT)
                ptm = pB1.tile([IND, 128], F32, tag="ptm")
                nc.tensor.matmul(ptm[:], st1r[0:IND, 1:IND + 1], W1f[:], start=True, stop=True)
                t1s = wp.tile([IND, 128], F32, tag="t1s")
                nc.vector.tensor_tensor(t1s[:], ptm[:], W1f[:], op=MUL)
                pep = pB1.tile([128, 1], F32, tag="pep")
                nc.tensor.matmul(pep[:], t1s[:], ones_f[0:IND, :], start=True, stop=True)
                nc.scalar.mul(Ep1, pep[:], 1.0 / NTOT)
            nc.vector.tensor_tensor(TA, Mu1, Mu1, op=MUL)
            nc.vector.tensor_tensor(Va1, Ep1, TA, op=SUB)
            nc.vector.tensor_scalar(TA, Va1, EPS, None, op0=ADD)
            nc.vector.reciprocal(TB, TA)
            nc.scalar.sqrt(TC, TB)
            nc.vector.tensor_tensor(A1, g1c, TC, op=MUL)
            nc.vector.tensor_tensor(TA, Mu1, A1, op=MUL)
            nc.vector.tensor_tensor(C1, be1c, TA, op=SUB)
            nc.vector.tensor_scalar(NA1, A1, -1.0, None, op0=MUL)
            nc.vector.tensor_scalar(NC1, C1, -1.0, None, op0=MUL)

            h1T = fmp.tile([128, GPC * 100], BF16, tag="fm")
            # ============ P6: y1 = agg1T^T W1 ; h1' = relu(x)+exp(min(x,0)) ============
            with tc.tile_pool(name="pY1", bufs=3, space="PSUM") as pY1:
                for cb in range(GPC // 5):
                    ps = pY1.tile([128, 500], F32, tag="ps")
                    nc.tensor.matmul(ps[:], W1b[:], agg1T[0:IND, cb * 500:(cb + 1) * 500],
                                     start=True, stop=True)
                    sl = slice(cb * 500, (cb + 1) * 500)
                    nc.scalar.activation(h1T[:, sl], ps[:], AF.Relu, bias=C1, scale=A1)
                    nc.scalar.activation(scr1[:, sl], ps[:], AF.Relu, bias=NC1, scale=NA1)
                    nc.scalar.activation(scr1[:, sl], scr1[:, sl], AF.Exp, scale=-1.0)
                    nc.vector.tensor_tensor(h1T[:, sl], h1T[:, sl], scr1[:, sl], op=ADD)

            if DBG:
                nc.vector.tensor_copy(dbg[0:IND, 781:981], agg1T[0:IND, 0:200])
                nc.vector.tensor_copy(dbg[:, 981:1181], h1T[:, 0:200])
            h1n = nmp.tile([100, GPC * 128], BF16, tag="nm")
            # ============ P7: transpose h1' to node-major; scale by invout ============
            with tc.tile_pool(name="pH1", bufs=2, space="PSUM") as pH1:
                for gb in range(GPC // 4):
                    ps = pH1.tile([100, 512], F32, tag="ps")
                    for gi in range(4):
                        g = gb * 4 + gi
                        nc.tensor.matmul(ps[0:100, gi * 128:(gi + 1) * 128],
                                         h1T[:, g * 100:(g + 1) * 100], identb[:],
                                         start=True, stop=True)
                    nc.scalar.copy(h1n[:, gb * 512:(gb + 1) * 512], ps[0:100, :])
                for g in range(GPC):
                    nc.vector.tensor_scalar(
                        h1n[:, g * 128:(g + 1) * 128], h1n[:, g * 128:(g + 1) * 128],
                        invout[:, g:g + 1], None, op0=MUL)

            agg2s = nmp.tile([100, GPC * 128], BF16, tag="nm")
            # ============ P8: conv2 MP + kappa ; z2 = agg2*invin - kap ============
            with tc.tile_pool(name="pC2", bufs=2, space="PSUM") as pC2:
                for gb in range(GPC // 4):
                    ps = pC2.tile([128, 512], F32, tag="ps")
                    for gi in range(4):
                        g = gb * 4 + gi
                        nc.tensor.matmul(ps[:, gi * 128:(gi + 1) * 128],
                                         AT[:, g * 128:(g + 1) * 128],
                                         h1n[:, g * 128:(g + 1) * 128],
                                         start=True, stop=True)
                    for gi in range(4):
                        g = gb * 4 + gi
                        nc.scalar.activation(
                            agg2s[:, g * 128:(g + 1) * 128],
                            ps[0:100, gi * 128:(gi + 1) * 128],
                            AF.Identity, scale=invin[:, g:g + 1],
                            bias=kaps[:, g:g + 1])

            if DBG:
                nc.vector.tensor_copy(dbg[0:100, 1181:1437], h1n[:, 0:256])
            agg2T = fmp.tile([128, GPC * 100], BF16, tag="fm")
            # ============ P9: stats2 + transpose2 ============
            with tc.tile_pool(name="pS2", bufs=1, space="PSUM") as pS2, \
                 tc.tile_pool(name="pT2", bufs=2, space="PSUM") as pT2:
                psz2 = pS2.tile([128, 1], F32, tag="psz2")
                pm22 = pS2.tile([128, 128], F32, tag="pm22")
                for gb in range(GPC // 5):
                    pt = pT2.tile([128, 500], F32, tag="pt")
                    for gi in range(5):
                        g = gb * 5 + gi
                        a = agg2s[:, g * 128:(g + 1) * 128]
                        nc.tensor.matmul(pt[:, gi * 100:(gi + 1) * 100], a,
                                         identb[0:100, 0:100], start=True, stop=True)
                        nc.tensor.matmul(psz2[:], a, ones_b[0:100, :],
                                         start=(g == 0), stop=(g == GPC - 1))
                        nc.tensor.matmul(pm22[:], a, a,
                                         start=(g == 0), stop=(g == GPC - 1))
                    nc.scalar.copy(agg2T[:, gb * 500:(gb + 1) * 500], pt[:])
                st2 = wp.tile([128, HID + 1], F32, tag="st2")
                nc.scalar.copy(st2[:, 0:1], psz2[:])
                nc.scalar.copy(st2[:, 1:HID + 1], pm22[:])
                nc.gpsimd.dma_start(d_ar2_i[:], st2[:])

            # ============ AR2 + BN2 coefficients ============
            nc.gpsimd.collective_compute(
                "AllReduce", ADD, replica_groups=RG,
                ins=[d_ar2_i.ap()], outs=[d_ar2_o.ap()])
            st2r = wp.tile([128, HID + 1], F32, tag="st2r")
            nc.gpsimd.dma_start(st2r[:], d_ar2_o[:])

            with tc.tile_pool(name="pB2", bufs=1, space="PSUM") as pB2:
                pmu = pB2.tile([128, 1], F32, tag="pmu")
                nc.tensor.matmul(pmu[:], W2f[:], st2r[:, 0:1], start=True, stop=True)
                nc.scalar.mul(Mu2, pmu[:], 1.0 / NTOT)
                ptm = pB2.tile([128, 128], F32, tag="ptm")
                nc.tensor.matmul(ptm[:], st2r[:, 1:HID + 1], W2f[:], start=True, stop=True)
                t2s = wp.tile([128, 128], F32, tag="t2s")
                nc.vector.tensor_tensor(t2s[:], ptm[:], W2f[:], op=MUL)
                pep = pB2.tile([128, 1], F32, tag="pep")
                nc.tensor.matmul(pep[:], t2s[:], ones_f[:], start=True, stop=True)
                nc.scalar.mul(Ep2, pep[:], 1.0 / NTOT)
            nc.vector.tensor_tensor(TA, Mu2, Mu2, op=MUL)
            nc.vector.tensor_tensor(Va2, Ep2, TA, op=SUB)
            nc.vector.tensor_scalar(TA, Va2, EPS, None, op0=ADD)
            nc.vector.reciprocal(TB, TA)
            nc.scalar.sqrt(TC, TB)
            nc.vector.tensor_tensor(A2, g2c, TC, op=MUL)
            nc.vector.tensor_tensor(TA, Mu2, A2, op=MUL)
            nc.vector.tensor_tensor(C2, be2c, TA, op=SUB)
            nc.vector.tensor_scalar(NA2, A2, -1.0, None, op0=MUL)
            nc.vector.tensor_scalar(NC2, C2, -1.0, None, op0=MUL)

            if DBG:
                nc.vector.tensor_copy(dbg[0:100, 1437:1693], agg2s[:, 0:256])
            h2T = fmp.tile([128, GPC * 100], BF16, tag="fm")
            # ============ P10: y2 + ELU -> h2' (feature-major) ============
            with tc.tile_pool(name="pY2", bufs=3, space="PSUM") as pY2:
                for cb in range(GPC // 5):
                    ps = pY2.tile([128, 500], F32, tag="ps")
                    nc.tensor.matmul(ps[:], W2b[:], agg2T[:, cb * 500:(cb + 1) * 500],
                                     start=True, stop=True)
                    sl = slice(cb * 500, (cb + 1) * 500)
                    nc.scalar.activation(h2T[:, sl], ps[:], AF.Relu, bias=C2, scale=A2)
                    nc.scalar.activation(scr1[:, sl], ps[:], AF.Relu, bias=NC2, scale=NA2)
                    nc.scalar.activation(scr1[:, sl], scr1[:, sl], AF.Exp, scale=-1.0)
                    nc.vector.tensor_tensor(h2T[:, sl], h2T[:, sl], scr1[:, sl], op=ADD)

            h2n = nmp.tile([100, GPC * 128], BF16, tag="nm")
            h2r = fmp.tile([100, GPC * 128], BF16, tag="fm")
            # ============ P11: readout ============
            # SWv = sum(Wv); broadcast to [128,1]
            with tc.tile_pool(name="pR", bufs=1, space="PSUM") as pR:
                psw = pR.tile([1, 1], F32, tag="psw")
                nc.tensor.matmul(psw[:], Wvf, ones_f[:], start=True, stop=True)
                swv_s = wp.tile([1, 1], F32, tag="swv_s")
                nc.scalar.copy(swv_s[:], psw[:])
                pswc = pR.tile([128, 1], F32, tag="pswc")
                nc.tensor.matmul(pswc[:], onesrow_f[:], swv_s[:], start=True, stop=True)
                swvc = wp.tile([128, 1], F32, tag="swvc")
                nc.scalar.copy(swvc[:], pswc[:])

                # q = (h2'^T) Wv per graph (node-major out), then transpose h2' (with -1)
                psq = pR.tile([100, 100], F32, tag="psq")
                with tc.tile_pool(name="pH2", bufs=2, space="PSUM") as pH2:
                    for gb in range(GPC // 4):
                        ps = pH2.tile([100, 512], F32, tag="ps")
                        for gi in range(4):
                            g = gb * 4 + gi
                            nc.tensor.matmul(ps[0:100, gi * 128:(gi + 1) * 128],
                                             h2T[:, g * 100:(g + 1) * 100], identb[:],
                                             start=True, stop=True)
                            nc.tensor.matmul(psq[:, g:g + 1],
                                             h2T[:, g * 100:(g + 1) * 100], Wvb[:],
                                             start=True, stop=True)
                        sl5 = slice(gb * 512, (gb + 1) * 512)
                        nc.scalar.copy(h2n[:, sl5], ps[0:100, :])
                        nc.vector.tensor_tensor(h2r[:, sl5], ps[0:100, :],
                                                h2n[:, sl5], op=SUB)
                # qt = (q - SWv) * invout
                nc.vector.tensor_scalar(qt, psq[:], swvc[0:100, :], None, op0=SUB)
                nc.vector.tensor_tensor(qt, qt, invout, op=MUL)
                nc.vector.tensor_copy(qtb[:], qt)

                # d = (A^T.T qt) * invin + bv
                psd2 = pR.tile([128, 100], F32, tag="psd2")
                for g in range(GPC):
                    nc.tensor.matmul(psd2[:, g:g + 1],
                                     AT[:, g * 128:(g + 1) * 128], qtb[:, g:g + 1],
                                     start=True, stop=True)
                nc.vector.tensor_tensor(tmpA, psd2[0:100, :], invin, op=MUL)
                nc.vector.tensor_scalar(tmpA, tmpA, bv_val, None, op0=ADD)
                nc.vector.tensor_copy(dbf[:], tmpA)

                # vec[f, g] = sum_n d_n h2[n, f]
                psv = pR.tile([128, 100], F32, tag="psv")
                for g in range(GPC):
                    nc.tensor.matmul(psv[:, g:g + 1],
                                     h2n[:, g * 128:(g + 1) * 128], dbf[:, g:g + 1],
                                     start=True, stop=False)
                    nc.tensor.matmul(psv[:, g:g + 1],
                                     h2r[:, g * 128:(g + 1) * 128], dbf[:, g:g + 1],
                                     start=False, stop=False)
                    nc.tensor.matmul(psv[:, g:g + 1],
                                     negones_b[:], dbf[:, g:g + 1],
                                     start=False, stop=True)
                vec_s2 = pp.tile([128, GPC], F32, tag="vec_s2")
                nc.scalar.copy(vec_s2[:], psv[:])
                nc.gpsimd.dma_start(d_ag_i[:], vec_s2[:])

            # ============ AG + replicated tail ============
            nc.gpsimd.collective_compute(
                "AllGather", mybir.AluOpType.bypass, replica_groups=RG,
                ins=[d_ag_i.ap()], outs=[d_ag_o.ap()])
            vecF = pp.tile([128, B], F32, tag="vecF")
            for c in range(NC_CORES):
                nc.sync.dma_start(vecF[:, c * GPC:(c + 1) * GPC],
                                  d_ag_o[c * 128:(c + 1) * 128, :])

            with tc.tile_pool(name="pZ", bufs=1, space="PSUM") as pZ:
                # global mean of vec
                cs = wp.tile([128, 1], F32, tag="cs")
                nc.vector.reduce_sum(cs[:], vecF[:], axis=mybir.AxisListType.X)
                ptot = pZ.tile([1, 1], F32, tag="ptot")
                nc.tensor.matmul(ptot[:], cs[:], ones_f[:], start=True, stop=True)
                tot_s = wp.tile([1, 1], F32, tag="tot_s")
                nc.scalar.copy(tot_s[:], ptot[:])
                pmu = pZ.tile([128, 1], F32, tag="pmu")
                nc.tensor.matmul(pmu[:], onesrow_f[:], tot_s[:], start=True, stop=True)
                nc.scalar.mul(MuV, pmu[:], -1.0 / (B * HID))   # negative mean

                # soft shrink + tanh
                r1 = pp.tile([128, B], F32, tag="r1")
                r2 = pp.tile([128, B], F32, tag="r2")
                nc.scalar.activation(r1[:], vecF[:], AF.Relu, bias=MuV, scale=1.0)
                nc.scalar.activation(r2[:], vecF[:], AF.Relu, bias=MuV, scale=-1.0)
                nc.vector.tensor_tensor(r1[:], r1[:], r2[:], op=SUB)
                hgT = pp.tile([128, B], BF16, tag="hgT")
                nc.scalar.activation(hgT[:], r1[:], AF.Tanh)

                # z3 = Wg^T hgT ; BN3 stats (local) ; tanh
                hg2T = pp.tile([128, B], BF16, tag="hg2T")
                sq_scr = pp.tile([128, B], BF16, tag="sq_scr")
                pz3 = pZ.tile([128, 400], F32, tag="pz3")
                pz3b = pZ.tile([128, 400], F32, tag="pz3b")
                nc.tensor.matmul(pz3[:], Wgb[:], hgT[:, 0:400], start=True, stop=True)
                nc.tensor.matmul(pz3b[:], Wgb[:], hgT[:, 400:800], start=True, stop=True)
                nc.vector.reduce_sum(TA, pz3[:], axis=mybir.AxisListType.X)
                nc.vector.reduce_sum(TB, pz3b[:], axis=mybir.AxisListType.X)
                nc.vector.tensor_tensor(Sz3, TA, TB, op=ADD)
                nc.scalar.activation(sq_scr[:, 0:400], pz3[:], AF.Square, accum_out=TA)
                nc.scalar.activation(sq_scr[:, 400:800], pz3b[:], AF.Square, accum_out=TB)
                nc.vector.tensor_tensor(Sq3, TA, TB, op=ADD)
                nc.vector.tensor_scalar(TA, Sz3, 1.0 / B, None, op0=MUL)      # mu3
                nc.vector.tensor_scalar(TB, Sq3, 1.0 / B, None, op0=MUL)      # E[z3^2]
                nc.vector.tensor_tensor(TC, TA, TA, op=MUL)
                nc.vector.tensor_tensor(TB, TB, TC, op=SUB)                   # var3
                nc.vector.tensor_scalar(TB, TB, EPS, None, op0=ADD)
                nc.vector.reciprocal(TC, TB)
                nc.scalar.sqrt(TB, TC)                                        # invstd3
                nc.vector.tensor_tensor(A3, g3c, TB, op=MUL)
                nc.vector.tensor_tensor(TC, TA, A3, op=MUL)
                nc.vector.tensor_tensor(C3, be3c, TC, op=SUB)
                nc.scalar.activation(hg2T[:, 0:400], pz3[:], AF.Tanh, bias=C3, scale=A3)
                nc.scalar.activation(hg2T[:, 400:800], pz3b[:], AF.Tanh, bias=C3, scale=A3)

                # out = tanh(hg2 Wc + bc)
                po1 = pZ.tile([NCLS, 400], F32, tag="po1")
                po2 = pZ.tile([NCLS, 400], F32, tag="po2")
                nc.tensor.matmul(po1[:], Wcb[:], hg2T[:, 0:400], start=True, stop=True)
                nc.tensor.matmul(po2[:], Wcb[:], hg2T[:, 400:800], start=True, stop=True)
                outT = pp.tile([NCLS, B], F32, tag="outT")
                nc.scalar.activation(outT[:, 0:400], po1[:], AF.Tanh, bias=bcc)
                nc.scalar.activation(outT[:, 400:800], po2[:], AF.Tanh, bias=bcc)

                # transpose per 100-graph block and store
                out_sb = pp.tile([100, NC_CORES * NCLS], F32, tag="out_sb")
                if DBG:
                    nc.vector.tensor_copy(dbg[0:100, 0:256], AT[:, 0:256])
                    nc.vector.tensor_copy(dbg[0:100, 256:356], indeg)
                    nc.vector.tensor_copy(dbg[0:100, 356:456], outdeg)
                    nc.vector.tensor_copy(dbg[0:100, 456:556], invin)
                    nc.vector.tensor_copy(dbg[0:100, 556:656], invout)
                    nc.vector.tensor_copy(dbg[0:IND, 736:757], st1r[0:IND, 0:IND+1])
                    nc.vector.tensor_copy(dbg[:, 757:781], col[:])
                    nc.vector.tensor_copy(dbg[:, 1693:1695], st2r[:, 0:2])
                    nc.vector.tensor_copy(dbg[:, 1695:1895], h2T[:, 0:200])
                    nc.vector.tensor_copy(dbg[0:100, 1895:1995], qt)
                    nc.vector.tensor_copy(dbg[0:100, 1995:2095], dbf[:])
                    nc.vector.tensor_copy(dbg[:, 2095:2195], vec_s2[:])
                    nc.vector.tensor_copy(dbg[:, 2195:2995], vecF[:])
                    nc.vector.tensor_copy(dbg[0:NCLS, 2995:3015], hgT[0:NCLS, 0:20])
                    nc.sync.dma_start(d_dbg[:], dbg[:])
                with tc.tile_pool(name="pO", bufs=2, space="PSUM") as pO:
                    for c in range(NC_CORES):
                        pso = pO.tile([100, NCLS], F32, tag="pso")
                        nc.tensor.matmul(pso[:], outT[:, c * 100:(c + 1) * 100],
                                         identf[0:NCLS, 0:NCLS],
                                         start=True, stop=True, is_transpose=True)
                        nc.scalar.copy(out_sb[:, c * NCLS:(c + 1) * NCLS], pso[:])
                        nc.sync.dma_start(d_out[c * 100:(c + 1) * 100, :],
                                          out_sb[:, c * NCLS:(c + 1) * NCLS])

    nc.compile()
    return nc


def kernel(**inputs):
    from concourse.bass_utils import run_bass_kernel_spmd

    src = np.asarray(inputs['src'])
    dst = np.asarray(inputs['dst'])
    bv_val = float(np.asarray(inputs['bv']).reshape(-1)[0])
    per_core = _host_prep(src, dst)

    Pm = np.zeros((128, 8), np.float32)
    Pm[:, 0] = np.asarray(inputs['g1'])
    Pm[:, 1] = np.asarray(inputs['be1'])
    Pm[:, 2] = np.asarray(inputs['g2'])
    Pm[:, 3] = np.asarray(inputs['be2'])
    Pm[:, 4] = np.asarray(inputs['g3'])
    Pm[:, 5] = np.asarray(inputs['be3'])
    Pm[0:NCLS, 6] = np.asarray(inputs['bc'])
    Pm[:, 7] = np.asarray(inputs['Wv'])[:, 0]

    nc = _build_program(bv_val)

    in_maps = []
    for c in range(NC_CORES):
        sl, dl = per_core[c]
        in_maps.append({
            "srcL": sl, "dstL": dl,
            "W1": np.asarray(inputs['W1'], np.float32),
            "W2": np.asarray(inputs['W2'], np.float32),
            "Wg": np.asarray(inputs['Wg'], np.float32),
            "Wc": np.asarray(inputs['Wc'], np.float32),
            "P": Pm,
        })
    trace = bool(int(os.environ.get("GNN_KERNEL_TRACE", "0")))
    res = run_bass_kernel_spmd(nc, in_maps, list(range(NC_CORES)), trace=trace)
    if trace:
        kernel.last_exec_time_ns = res.exec_time_ns
        kernel.last_results = res
    kernel.last_dbg = np.asarray(res.results[0].get("dbg")) if "dbg" in res.results[0] else None
    return np.asarray(res.results[0]["out"])

